# revision 24
# baseline (speedup 1.0000x reference)
"""AttentionNet (BiDAF-style) Trainium2 Bass kernel.

KEY STRUCTURE (faithful to the reference): every GRU scans over axis 0 of a
batch-first tensor — i.e. the recurrence runs over B=32 steps, while the
T=400 positions (and J=30 query positions) are independent lanes.

Sharding: the 400 context lanes are split 50/core across 8 cores; the 30
query lanes are replicated (cheap, and every core needs full Q for the
attention). Params replicated. Per-core compute is feature-major fp16 with
fp32 PSUM.

v3 (vs v2): all parameters packed into TWO DRAM inputs (pk16/pk32) and ONE
output — per-call dispatch overhead through the PJRT/axon path scales with
the argument count (~70us/tensor/call), so 26 inputs -> 2 is the single
biggest lever on measured per-call time. Cell micro-opts: one merged sigmoid
per step-dir, (z-1)*n via scalar_tensor_tensor (drops the 1-z op),
tensor_tensor_reduce fusions for q2c and gxc/gsum.
"""
import numpy as np
import ml_dtypes

import concourse.bass as bass
import concourse.mybir as mybir
import concourse.tile as tile
from concourse.bass_utils import run_bass_kernel_spmd

F32 = mybir.dt.float32
BF16 = mybir.dt.float16  # fp16: same PE speed as bf16, 10-bit mantissa
AF = mybir.ActivationFunctionType
ALU = mybir.AluOpType
AX = mybir.AxisListType
BF = np.float16

B_TOT, T, J, ANS = 32, 400, 30, 400
NB = 32              # scan steps (batch dim of the reference)
NCORES = 8
LN = T // NCORES     # 50 context lanes per core
JW = 32              # padded q-lane width (30 + 2 zeros)
W_CQ = LN + JW       # 82: combined ctx+q lane width in the ctx layer
KSHIFT = 33.0        # constant softmax shift; exact (cancels), S<=39.9

CFG = {"ctx": dict(kin=2, kc=2), "mod": dict(kin=16, kc=2), "p2g": dict(kin=4, kc=4)}


def _pk_layout():
    L16 = [("x_all", (128, 2 * NB * W_CQ))]
    for name in CFG:
        kin, kc = CFG[name]["kin"], CFG[name]["kc"]
        gc = 3 * kc
        for d in (0, 1):
            L16.append((f"{name}_wih{d}", (128, kin * gc * 128)))
        L16.append((f"{name}_whh", (128, 2 * kc * gc * 128)))
        L16.append((f"{name}_bhnr", (1, 2 * kc * 128)))
    L16 += [("p1_wT", (128, 20 * ANS)), ("p2_wT", (128, 24 * ANS)),
            ("p1_bd", (1, ANS)), ("p2_bd", (1, ANS)), ("ident", (128, 128))]
    L32 = [(f"{name}_gib", (128, 2 * 3 * CFG[name]["kc"])) for name in CFG]
    L32.append(("w123", (128, 12)))

    def offs(L):
        d, off = {}, 0
        for k, shp in L:
            n = int(np.prod(shp))
            d[k] = (off, shp)
            off += n
        return d, off

    D16, N16 = offs(L16)
    D32, N32 = offs(L32)
    return L16, D16, N16, L32, D32, N32


PK_L16, PK_D16, PK_N16, PK_L32, PK_D32, PK_N32 = _pk_layout()

_uid = [0]

def _split_excess_waits(nc, max_waits=1):
    for func in nc.m.functions:
        for block in func.blocks:
            new_insts = []
            for inst in block.instructions:
                si = inst.sync_info
                if si is not None and si.on_wait and len(si.on_wait) > max_waits:
                    waits = list(si.on_wait)
                    excess, keep = waits[:-max_waits], waits[-max_waits:]
                    for i in range(0, len(excess), max_waits):
                        chunk = excess[i:i + max_waits]
                        _uid[0] += 1
                        new_insts.append(mybir.InstNoOp(
                            name=f"waitsplit_nop_{_uid[0]}", ins=[], outs=[],
                            engine=inst.engine,
                            sync_info=mybir.SyncInfo(on_wait=list(chunk), on_update=[])))
                    inst.sync_info = mybir.SyncInfo(on_wait=list(keep),
                                                    on_update=list(si.on_update or []))
                new_insts.append(inst)
            block.instructions[:] = new_insts


def build_nc(taps=()):
    nc = bass.Bass()
    RG = [list(range(NCORES))]

    pk16 = nc.dram_tensor("pk16", [PK_N16], BF16, kind="ExternalInput")
    pk32 = nc.dram_tensor("pk32", [PK_N32], F32, kind="ExternalInput")
    out2 = nc.dram_tensor("out2", [2 * NB * ANS], F32, kind="ExternalOutput")

    def v16(key):
        off, shp = PK_D16[key]
        return pk16[off:off + int(np.prod(shp))].rearrange(
            "(p x) -> p x", p=shp[0])

    def v32(key):
        off, shp = PK_D32[key]
        return pk32[off:off + int(np.prod(shp))].rearrange(
            "(p x) -> p x", p=shp[0])

    ncop = [0]
    def spread_copy(out, in_, bias=None):
        ncop[0] += 1
        if bias is not None:
            if ncop[0] % 2 == 0:
                nc.scalar.activation(out, in_, AF.Identity, bias=bias)
            else:
                nc.vector.tensor_scalar(out, in_, bias, None, op0=ALU.add)
        else:
            if ncop[0] % 2 == 0:
                nc.scalar.copy(out, in_)
            else:
                nc.vector.tensor_copy(out, in_)

    lp = nc.allow_low_precision("fp16 lane-sums are within tolerance")
    lp.__enter__()

    with tile.TileContext(nc) as tc:
      with tc.tile_pool(name="const", bufs=1) as constp, \
           tc.tile_pool(name="acts", bufs=1) as acts, \
           tc.tile_pool(name="ccdram", bufs=1, space="DRAM") as ccd:

        ident = constp.tile([128, 128], BF16, tag="ident")
        nc.sync.dma_start(ident[:], v16("ident"))
        ones_row = constp.tile([1, ANS], BF16, tag="ones_row")
        nc.vector.memset(ones_row[:], 1.0)

        from contextlib import ExitStack
        _escq = ExitStack()
        pcq = _escq.enter_context(tc.tile_pool(name="pCQ", bufs=1))
        CQ = pcq.tile([128, 4, NB + 2, W_CQ], BF16, tag="CQ")
        M = acts.tile([128, 4, NB + 2, LN], BF16, tag="M")
        # fp32: logits reach O(1e4); fp16 partial sums (ulp 8 at 1.3e4) can
        # flip softmax argmax on close rows (row-6 top-gap is ~3.4 logits)
        gsum = acts.tile([128, 16, NB], F32, tag="gsum")
        msum = acts.tile([128, 4, NB], F32, tag="msum")
        m2sum = acts.tile([128, 8, NB], F32, tag="m2sum")
        p1part = acts.tile([NB, ANS], F32, tag="p1part")
        p2part = acts.tile([NB, ANS], F32, tag="p2part")
        # zero h0 boundary rows only
        for tl, wd, kk in ((CQ, W_CQ, 4), (M, LN, 4)):
            nc.vector.memset(tl[:, :, 0, :], 0.0)
            nc.vector.memset(tl[:, :, NB + 1, :], 0.0)

        def tap(name, src):
            if name in taps:
                to = nc.dram_tensor(f"tap_{name}", list(src.shape), src.dtype,
                                    kind="ExternalOutput")
                nc.sync.dma_start(to[:], src[:])

        # -------- gi precompute --------
        # ks=(lo,hi) accumulates only input chunks lo..hi-1; restore=True
        # starts from the previously spilled fp16 partial in gi_tile (via an
        # identity matmul) so a later pass can add the remaining chunks.
        def gi_phase(name, x_mov, width, gi_tile, psp, gib_sb, wih_views,
                     dirs=(0, 1), bset=None, ks=None, restore=False,
                     add_bias=True):
            kin, kc = CFG[name]["kin"], CFG[name]["kc"]
            gc = 3 * kc
            if ks is None:
                ks = (0, kin)
            bchunk = max(1, 512 // width)
            allb = list(range(0, NB, bchunk))
            if bset is None:
                bset = allb
            for d in dirs:
                wv = wih_views[d]
                for b0 in bset:
                    nb = min(bchunk, NB - b0)
                    for g in range(gc):
                        pt = psp.tile([128, 512], F32, tag="gi")
                        dst = gi_tile[:, d, g, b0:b0 + nb, :]
                        if restore:
                            nc.tensor.matmul(pt[:, :nb * width], ident[:], dst,
                                             start=True, stop=False,
                                             skip_group_check=True)
                        for k in range(*ks):
                            nc.tensor.matmul(pt[:, :nb * width], wv[:, k, g, :],
                                             x_mov(k)[:, b0:b0 + nb, :],
                                             start=(k == ks[0] and not restore),
                                             stop=(k == ks[1] - 1),
                                             skip_group_check=True)
                        spread_copy(
                            dst,
                            pt[:, :nb * width].rearrange("p (b w) -> p b w", w=width),
                            bias=gib_sb[:, d, g:g + 1] if add_bias else None)

        def load_wih(name, pool, d):
            kin, kc = CFG[name]["kin"], CFG[name]["kc"]
            gc = 3 * kc
            wih_sb = pool.tile([128, kin * gc * 128], BF16, tag=f"{name}_wih{d}")
            nc.sync.dma_start(wih_sb[:], v16(f"{name}_wih{d}"))
            return wih_sb[:].rearrange("p (a g n) -> p a g n", a=kin, n=128)

        def load_gib(name, pool):
            gc = 3 * CFG[name]["kc"]
            gib_sb = pool.tile([128, 2, gc], F32, tag=f"{name}_gib")
            nc.sync.dma_start(gib_sb[:].rearrange("p d g -> p (d g)"),
                              v32(f"{name}_gib"))
            return gib_sb

        def load_whh(name, pool):
            kc = CFG[name]["kc"]
            gc = 3 * kc
            whh_sb = pool.tile([128, 2, kc * gc * 128], BF16, tag=f"{name}_whh")
            nc.sync.dma_start(whh_sb[:].rearrange("p d x -> p (d x)"),
                              v16(f"{name}_whh"))
            bhnr_sb = pool.tile([1, 2, kc, 128], BF16, tag=f"{name}_bhnr")
            nc.sync.dma_start(
                bhnr_sb[:].rearrange("p d k n -> p (d k n)"),
                v16(f"{name}_bhnr"))
            return whh_sb[:].rearrange("p d (a g n) -> p d a g n", a=kc, n=128), bhnr_sb

        # -------- recurrence (fused cell) --------
        # bhn broadcast [128, 2, kc, width], materialized once per layer so
        # the per-step pgn init is ONE ident matmul instead of kc rank-1
        # matmuls (each rank-1 pays a full 128-col LDWEIGHTS on HW)
        def make_bhn_bc(name, pool, psp, bhnr_sb, width):
            kc = CFG[name]["kc"]
            bc = pool.tile([128, 2, kc, width], BF16, tag=f"{name}bhbc")
            for d in (0, 1):
                pt = psp.tile([128, 512], F32, tag="gi")
                for g in range(kc):
                    nc.tensor.matmul(pt[:, g * width:(g + 1) * width],
                                     bhnr_sb[:, d, g, :], ones_row[:, 0:width],
                                     start=(g == 0), stop=(g == kc - 1),
                                     skip_group_check=True)
                spread_copy(bc[:, d, :, :],
                            pt[:, :kc * width].rearrange("p (g w) -> p g w",
                                                         w=width))
            return bc

        def recur_step(name, whh_v, bhn_bc, gi_tile, out_tile, width, psp,
                       cellp, d, b):
            kc = CFG[name]["kc"]
            gd = d
            koff = 0 if d == 0 else kc
            rd, wr = (b, b + 1) if d == 0 else (b + 2, b + 1)
            h_prev = out_tile[:, koff:koff + kc, rd, :]
            pgrz = psp.tile([128, 2 * kc * width], F32, tag=f"{name}rz{d}")
            pgn = psp.tile([128, kc * width], F32, tag=f"{name}n{d}")
            # PE: gi(rz) copy, r tiles, z tiles, bhn copy, n tiles
            nc.tensor.matmul(pgrz[:], ident[:], gi_tile[:, gd, 0:2 * kc, b, :],
                             start=True, stop=False, skip_group_check=True)
            for g in range(2 * kc):
                for k in range(kc):
                    nc.tensor.matmul(pgrz[:, g * width:(g + 1) * width],
                                     whh_v[:, d, k, g, :], h_prev[:, k, :],
                                     start=False, stop=False,
                                     skip_group_check=True)
            # start=True only on the first matmul touching this 2KB PSUM
            # region: start marks the WHOLE zero-region pending, so a second
            # start would wipe the previous slice's bias on the next write.
            nc.tensor.matmul(pgn[:], ident[:], bhn_bc[:, d, :, :],
                             start=True, stop=False, skip_group_check=True)
            for g in range(kc):
                for k in range(kc):
                    nc.tensor.matmul(pgn[:, g * width:(g + 1) * width],
                                     whh_v[:, d, k, 2 * kc + g, :], h_prev[:, k, :],
                                     start=False,
                                     stop=(g == kc - 1 and k == kc - 1),
                                     skip_group_check=True)
            # ctx cell intermediates in fp32: C's accuracy bounds the head
            # logits (|c2q|~2e3 amplifies C's relative error into G)
            CT = F32 if name == "ctx" else BF16
            rz = cellp.tile([128, 2 * kc, width], CT, tag=f"{name}rz{d}s")
            nc.scalar.activation(
                rz[:], pgrz[:].rearrange("p (g w) -> p g w", w=width),
                AF.Sigmoid)
            t1 = cellp.tile([128, kc, width], CT, tag=f"{name}t1{d}")
            nc.vector.tensor_tensor(
                t1[:], pgn[:].rearrange("p (g w) -> p g w", w=width),
                rz[:, 0:kc, :], op=ALU.mult)
            npre = cellp.tile([128, kc, width], CT, tag=f"{name}np{d}")
            nc.vector.tensor_tensor(npre[:], t1[:], gi_tile[:, gd, 2 * kc:, b, :],
                                    op=ALU.add)
            wz = cellp.tile([128, kc, width], CT, tag=f"{name}w{d}")
            nc.gpsimd.tensor_tensor(wz[:], rz[:, kc:2 * kc, :], h_prev, op=ALU.mult)
            nt = cellp.tile([128, kc, width], CT, tag=f"{name}nt{d}")
            nc.scalar.activation(nt[:], npre[:], AF.Tanh)
            zm = cellp.tile([128, kc, width], CT, tag=f"{name}m1{d}")
            # (z - 1) * n, then h' = z*h - (z-1)*n
            nc.vector.scalar_tensor_tensor(zm[:], rz[:, kc:2 * kc, :], 1.0,
                                           nt[:], op0=ALU.subtract,
                                           op1=ALU.mult)
            nc.vector.tensor_tensor(out_tile[:, koff:koff + kc, wr, :],
                                    wz[:], zm[:], op=ALU.subtract)

        def recur_steps(name, whh_v, bhn_bc, gi_tile, out_tile, width, psp,
                        cellp, srange, dirs=(0, 1)):
            for s in srange:
                for d in dirs:
                    recur_step(name, whh_v, bhn_bc, gi_tile, out_tile, width,
                               psp, cellp, d, s if d == 0 else NB - 1 - s)

        def allreduce(sb_aps, op):
            tot = sum(int(np.prod(a.shape)) for a in sb_aps)
            _uid[0] += 1
            cin = ccd.tile([tot], F32, tag=f"cc_in{_uid[0]}", bufs=1)
            cout = ccd.tile([tot], F32, tag=f"cc_out{_uid[0]}", bufs=1)
            off = 0
            for a in sb_aps:
                n = int(np.prod(a.shape))
                nc.sync.dma_start(
                    cin[off:off + n].rearrange("(p f) -> p f", p=a.shape[0]), a)
                off += n
            nc.gpsimd.collective_compute("AllReduce", op, replica_groups=RG,
                                         ins=[cin.opt()], outs=[cout.opt()])
            off = 0
            for a in sb_aps:
                n = int(np.prod(a.shape))
                nc.sync.dma_start(
                    a, cout[off:off + n].rearrange("(p f) -> p f", p=a.shape[0]))
                off += n

        # ================= ctx layer (gi interleaved with recurrence) ========
        with tc.tile_pool(name="pctx", bufs=1) as pctx:
            xs = pctx.tile([128, 2, NB, W_CQ], BF16, tag="xs")
            nc.sync.dma_start(xs[:].rearrange("p d b w -> p (d b w)"),
                              v16("x_all"))
            gi_ctx = pctx.tile([128, 2, 6, NB, W_CQ], BF16, tag="gi_ctx")
            gib_sb = load_gib("ctx", pctx)
            wihv = [load_wih("ctx", pctx, d) for d in (0, 1)]
            whh_v, bhnr_sb = load_whh("ctx", pctx)
            x_mov = lambda k: xs[:, k, :, :]
            with tc.tile_pool(name="psbh_ctx", bufs=1, space="PSUM") as psb:
                bc_ctx = make_bhn_bc("ctx", pctx, psb, bhnr_sb, W_CQ)
            with tc.tile_pool(name="psgi_ctx", bufs=3, space="PSUM") as psp, \
                 tc.tile_pool(name="psrec_ctx", bufs=1, space="PSUM") as psr, \
                 tc.tile_pool(name="cell_ctx", bufs=3) as cellp:
                # chunk starts for bchunk=6: 0,6,12,18,24,30
                gi_phase("ctx", x_mov, W_CQ, gi_ctx, psp, gib_sb, wihv,
                         bset=[0, 30, 24])
                recur_steps("ctx", whh_v, bc_ctx, gi_ctx, CQ, W_CQ, psr,
                            cellp, range(0, 6))
                gi_phase("ctx", x_mov, W_CQ, gi_ctx, psp, gib_sb, wihv,
                         bset=[6, 18])
                recur_steps("ctx", whh_v, bc_ctx, gi_ctx, CQ, W_CQ, psr,
                            cellp, range(6, 12))
                gi_phase("ctx", x_mov, W_CQ, gi_ctx, psp, gib_sb, wihv,
                         bset=[12])
                recur_steps("ctx", whh_v, bc_ctx, gi_ctx, CQ, W_CQ, psr,
                            cellp, range(12, NB))
            tap("gi_ctx", gi_ctx)
        tap("CQ", CQ)

        # ================= attention =================
        with tc.tile_pool(name="pattn", bufs=1) as pa:
            from contextlib import ExitStack
            _esat = ExitStack()
            patmp = _esat.enter_context(tc.tile_pool(name="patmp", bufs=1))
            wv = constp.tile([128, 4, 3], F32, tag="wv")
            nc.sync.dma_start(wv[:].rearrange("p a b -> p (a b)"), v32("w123"))
            w1b = constp.tile([128, 4, 1], BF16, tag="w1b")
            nc.vector.tensor_copy(w1b[:], wv[:, :, 0:1])
            w2b = constp.tile([128, 4, 1], BF16, tag="w2b")
            nc.vector.tensor_copy(w2b[:], wv[:, :, 1:2])

            q3 = patmp.tile([128, 4, NB, JW], BF16, tag="q3")
            nc.vector.memset(q3[:], 0.0)
            for ch in range(4):
                nc.vector.tensor_scalar(q3[:, ch, :, 0:J],
                                        CQ[:, ch, 1:NB + 1, LN:LN + J],
                                        wv[:, ch, 2:3], None, op0=ALU.mult)

            lsum1 = pa.tile([1, NB], F32, tag="lsum1")
            q2c = pa.tile([128, 4, NB], F32, tag="q2c")
            e_bc = pa.tile([128, NB, LN], BF16, tag="e_bc")
            c2q = pa.tile([128, 4, NB, LN], BF16, tag="c2q")
            gxc = pa.tile([128, 4, NB, LN], BF16, tag="gxc")
            gxq = pa.tile([128, 4, NB, LN], BF16, tag="gxq")
            q2cn16 = pa.tile([128, 4, NB], BF16, tag="q2cn16")

            with tc.tile_pool(name="psattn", bufs=2, space="PSUM") as psa:
                # cw1[i] = sum_ch C.w1 ; qw2p[j] = sum_ch Q.w2
                cw1 = patmp.tile([1, NB, LN], BF16, tag="cw1")
                for b0 in range(0, NB, 10):
                    nb = min(10, NB - b0)
                    pc = psa.tile([1, 512], F32, tag="psA")
                    for k in range(4):
                        nc.tensor.matmul(pc[:, :nb * LN], w1b[:, k, :],
                                         CQ[:, k, b0 + 1:b0 + 1 + nb, 0:LN],
                                         start=(k == 0), stop=(k == 3))
                    nc.vector.tensor_copy(
                        cw1[:, b0:b0 + nb, :],
                        pc[:, :nb * LN].rearrange("p (b w) -> p b w", w=LN))
                qw2p = patmp.tile([1, NB, JW], BF16, tag="qw2p")
                nc.vector.memset(qw2p[:], 0.0)
                for b0 in range(0, NB, 16):
                    pq = psa.tile([1, 512], F32, tag="psA")
                    for k in range(4):
                        nc.tensor.matmul(pq[:, :16 * J], w2b[:, k, :],
                                         CQ[:, k, b0 + 1:b0 + 17, LN:LN + J],
                                         start=(k == 0), stop=(k == 3))
                    nc.vector.tensor_copy(
                        qw2p[:, b0:b0 + 16, 0:J],
                        pq[:, :16 * J].rearrange("p (b w) -> p b w", w=J))

                # S^T [j, b, w] in b-groups of 10 per PSUM bank
                s_sbT = patmp.tile([32, NB, LN], BF16, tag="s_sbT")
                for b0 in range(0, NB, 10):
                    nb = min(10, NB - b0)
                    psT = psa.tile([32, 512], F32, tag="psT")
                    for i in range(nb):
                        b = b0 + i
                        sl = psT[:, i * LN:(i + 1) * LN]
                        for k in range(4):
                            nc.tensor.matmul(sl, q3[:, k, b, :],
                                             CQ[:, k, b + 1, 0:LN],
                                             start=(k == 0), stop=False,
                                             skip_group_check=True)
                        nc.tensor.matmul(psT[0:J, i * LN:(i + 1) * LN],
                                         ones_row[:, 0:J], cw1[:, b, :],
                                         start=False, stop=False,
                                         skip_group_check=True)
                        nc.tensor.matmul(sl, qw2p[:, b, :], ones_row[:, 0:LN],
                                         start=False, stop=True,
                                         skip_group_check=True)
                    spread_copy(
                        s_sbT[:, b0:b0 + nb, :],
                        psT[:, :nb * LN].rearrange("p (b w) -> p b w", w=LN))

                # smax over j (partitions 0..29) -> exp(K-shift) -> broadcast
                smax1 = patmp.tile([1, NB, LN], F32, tag="smax1")
                nc.gpsimd.tensor_reduce(
                    smax1[:].rearrange("p b w -> p (b w)"),
                    s_sbT[0:J, :, :].rearrange("p b w -> p (b w)"),
                    axis=AX.C, op=ALU.max)
                e1 = patmp.tile([1, NB, LN], BF16, tag="e1")
                nks = patmp.tile([1, 1], F32, tag="nks")
                nc.vector.memset(nks[:], -KSHIFT)
                nc.scalar.activation(e1[:], smax1[:], AF.Exp, bias=nks[:])
                nc.vector.tensor_reduce(lsum1[:], e1[:], axis=AX.X, op=ALU.add)
                e_d = ccd.tile([NB * LN], BF16, tag="e_d", bufs=1)
                nc.sync.dma_start(
                    e_d[:].rearrange("(p f) -> p f", p=1),
                    e1[:].rearrange("p b w -> p (b w)"))
                nc.sync.dma_start(
                    e_bc[:].rearrange("p b w -> p (b w)"),
                    bass.AP(tensor=e_d.tensor, offset=e_d.offset,
                            ap=[[0, 128], [1, NB * LN]]))

                # q2c partials: wide mult + reduce per ch
                for ch in range(4):
                    tmp = patmp.tile([128, NB, LN], BF16, tag="q2ctmp")
                    nc.vector.tensor_tensor(tmp[:], CQ[:, ch, 1:NB + 1, 0:LN],
                                            e_bc[:], op=ALU.mult)
                    nc.vector.tensor_reduce(q2c[:, ch, :], tmp[:], axis=AX.X,
                                            op=ALU.add)

                # ---- AllReduce #1: lsum + q2c (hidden under c2q/gxc below)
                allreduce([lsum1[:], q2c[:].rearrange("p a b -> p (a b)")],
                          ALU.add)

                # c2q[feat, b, w] = sum_j Q^T[j, feat] * S^T[j, b, w]
                # c2q/gxc reach O(1e3-1e4): G sums for the head logits are
                # taken from fp32 copies (fp16 element rounding aggregated
                # over lanes is too coarse for close softmax rows); the gi
                # matmuls use the fp16 copies.
                qbm = patmp.tile([32, NB, 128], BF16, tag="qbm")
                for ch in range(4):
                    for g in range(8):
                        ptq = psa.tile([32, 4, 128], BF16, tag="ptq")
                        for i in range(4):
                            nc.tensor.transpose(
                                ptq[:, i, :], CQ[:, ch, 1 + 4 * g + i, LN:LN + JW],
                                ident[:])
                        spread_copy(qbm[:, 4 * g:4 * g + 4, :], ptq[:])
                    c2q32 = patmp.tile([128, NB, LN], F32, tag="c2q32", bufs=2)
                    for bg in range(4):
                        pc2 = psa.tile([128, 400], F32, tag="pc2")
                        for i in range(8):
                            b = bg * 8 + i
                            nc.tensor.matmul(
                                pc2[:, i * LN:(i + 1) * LN],
                                qbm[:, b, :],
                                s_sbT[:, b, :], start=True, stop=True,
                                skip_group_check=True)
                        spread_copy(
                            c2q32[:, bg * 8:(bg + 1) * 8, :],
                            pc2[:].rearrange("p (b w) -> p b w", w=LN))
                    nc.gpsimd.tensor_copy(c2q[:, ch, :, :], c2q32[:])
                    nc.vector.tensor_reduce(gsum[:, 4 + ch, :], c2q32[:],
                                            axis=AX.X, op=ALU.add)
                    gxc32 = patmp.tile([128, NB, LN], F32, tag="gxc32", bufs=2)
                    nc.vector.tensor_tensor(gxc32[:], CQ[:, ch, 1:NB + 1, 0:LN],
                                            c2q32[:], op=ALU.mult)
                    nc.vector.tensor_reduce(gsum[:, 8 + ch, :], gxc32[:],
                                            axis=AX.X, op=ALU.add)
                    nc.gpsimd.tensor_copy(gxc[:, ch, :, :], gxc32[:])

            def gpart(k):
                if k < 4:
                    return CQ[:, k, 1:NB + 1, 0:LN]
                if k < 8:
                    return c2q[:, k - 4, :, :]
                if k < 12:
                    return gxc[:, k - 8, :, :]
                return gxq[:, k - 12, :, :]

            for k in range(4):
                nc.vector.tensor_reduce(gsum[:, k, :], gpart(k),
                                        axis=AX.X, op=ALU.add)

            # ---- post-collective: q2cn, gxq, gsum[12:16] (AR#1 hidden by
            # the c2q transpose/matmul + gxc work above)
            rs1 = pa.tile([1, NB], F32, tag="rs1")
            nc.vector.reciprocal(rs1[:], lsum1[:])
            rs_bc = pa.tile([128, NB], F32, tag="rs_bc")
            rs_d = ccd.tile([NB], F32, tag="rs_d", bufs=1)
            nc.sync.dma_start(rs_d[:].rearrange("(p f) -> p f", p=1),
                              rs1[:])
            nc.sync.dma_start(
                rs_bc[:],
                bass.AP(tensor=rs_d.tensor, offset=rs_d.offset,
                        ap=[[0, 128], [1, NB]]))
            q2cn = pa.tile([128, 4, NB], F32, tag="q2cn")
            nc.vector.tensor_tensor(
                q2cn[:], q2c[:],
                bass.AP(tensor=rs_bc.tensor, offset=rs_bc.offset,
                        ap=[rs_bc.ap[0], [0, 4], rs_bc.ap[1]]),
                op=ALU.mult)
            nc.vector.tensor_copy(q2cn16[:], q2cn[:])
            nc.vector.tensor_tensor(gsum[:, 12:16, :], gsum[:, 0:4, :],
                                    q2cn[:], op=ALU.mult)
            for ch in range(4):
                qs = q2cn16[:, ch, :]
                nc.vector.tensor_tensor(
                    gxq[:, ch, :, :], CQ[:, ch, 1:NB + 1, 0:LN],
                    bass.AP(tensor=qs.tensor, offset=qs.offset,
                            ap=[qs.ap[0], qs.ap[1], [0, LN]]),
                    op=ALU.mult)

            tap("c2q", c2q)
            _esat.close()

            # ================= mod layer (single gi pass, 16 chunks) ========
            with tc.tile_pool(name="pmod", bufs=1) as pm:
                gi_mod = pm.tile([128, 2, 6, NB, LN], BF16, tag="gi_mod")
                with tc.tile_pool(name="pmodw", bufs=1) as pmw:
                    gib_m = load_gib("mod", pmw)
                    wv_m = [load_wih("mod", pmw, d) for d in (0, 1)]
                    with tc.tile_pool(name="psgi_mod", bufs=4,
                                      space="PSUM") as psp:
                        for d in (0, 1):
                            gi_phase("mod", gpart, LN, gi_mod, psp, gib_m,
                                     wv_m, dirs=(d,))
                whh_v, bhnr_sb = load_whh("mod", pm)
                with tc.tile_pool(name="psbh_mod", bufs=1, space="PSUM") as psb:
                    bc_mod = make_bhn_bc("mod", pm, psb, bhnr_sb, LN)
                with tc.tile_pool(name="psrec_mod", bufs=2, space="PSUM") as psr, \
                     tc.tile_pool(name="cell_mod", bufs=3) as cellp:
                    recur_steps("mod", whh_v, bc_mod, gi_mod, M, LN, psr,
                                cellp, range(NB))
        _escq.close()
        tap("gsum", gsum)
        tap("M", M)

        # ================= p1 partial logits + p2g =================
        with tc.tile_pool(name="phead", bufs=1) as ph:
            for k in range(4):
                nc.vector.tensor_reduce(msum[:, k, :], M[:, k, 1:NB + 1, :],
                                        axis=AX.X, op=ALU.add)
            tap("msum", msum)

            def head_partial(wkey, bkey, wtot, c0, srcs, dst, pstag):
                # logits for weight chunks [c0, c0+n) of the packed head
                # weight (wtot chunks); bias included iff bkey is not None.
                nchunk = sum(s.shape[1] for s in srcs)
                woff, wshp = PK_D16[wkey]
                wbase = pk16[woff:woff + int(np.prod(wshp))].rearrange(
                    "(p x) -> p x", p=128)
                with tc.tile_pool(name=f"phw_{pstag}", bufs=1) as phw:
                    w_sb = phw.tile([128, nchunk, ANS], BF16,
                                    tag=f"w_head{pstag}")
                    nc.sync.dma_start(
                        w_sb[:].rearrange("p c a -> p (c a)"),
                        wbase[:, c0 * ANS:(c0 + nchunk) * ANS])
                    if bkey is not None:
                        b_sb = phw.tile([1, ANS], BF16, tag=f"b_head{pstag}")
                        nc.sync.dma_start(b_sb[:], v16(bkey))
                    # hi/lo split: fp32 sums round to fp16 for the PE, the
                    # fp16 residual rides a second matmul — keeps the O(1e4)
                    # logits accurate to ~fp32 through the head.
                    gm = phw.tile([128, nchunk, NB], BF16, tag=f"gm_{pstag}")
                    gml = phw.tile([128, nchunk, NB], BF16, tag=f"gml_{pstag}")
                    off = 0
                    for s in srcs:
                        nchk = s.shape[1]
                        nc.vector.tensor_copy(gm[:, off:off + nchk, :], s[:])
                        nc.vector.tensor_tensor(gml[:, off:off + nchk, :],
                                                s[:], gm[:, off:off + nchk, :],
                                                op=ALU.subtract)
                        off += nchk
                    with tc.tile_pool(name=f"psh_{pstag}", bufs=1,
                                      space="PSUM") as psh:
                        ps_ = psh.tile([NB, ANS], F32, tag=f"ps{pstag}")
                        if bkey is not None:
                            nc.tensor.matmul(ps_[:], ones_row[:, 0:NB], b_sb[:],
                                             start=True, stop=False,
                                             skip_group_check=True)
                        for k in range(nchunk):
                            nc.tensor.matmul(ps_[:], gm[:, k, :], w_sb[:, k, :],
                                             start=(bkey is None and k == 0),
                                             stop=False,
                                             skip_group_check=True)
                        for k in range(nchunk):
                            nc.tensor.matmul(ps_[:], gml[:, k, :], w_sb[:, k, :],
                                             start=False, stop=(k == nchunk - 1),
                                             skip_group_check=True)
                        nc.vector.tensor_copy(dst[:], ps_[:])

            def head_post(src, out_ap, pstag):
                mx = ph.tile([NB, 1], F32, tag=f"mx{pstag}")
                nc.vector.tensor_reduce(mx[:], src[:], axis=AX.X, op=ALU.max)
                nmx = ph.tile([NB, 1], F32, tag=f"nmx{pstag}")
                nc.vector.tensor_scalar_mul(nmx[:], mx[:], -1.0)
                sm = ph.tile([NB, 1], F32, tag=f"sm{pstag}")
                ee = ph.tile([NB, ANS], F32, tag=f"e{pstag}")
                nc.scalar.activation(ee[:], src[:], AF.Exp, bias=nmx[:],
                                     accum_out=sm[:])
                rr = ph.tile([NB, 1], F32, tag=f"r{pstag}")
                nc.vector.reciprocal(rr[:], sm[:])
                po = ph.tile([NB, ANS], F32, tag=f"po{pstag}")
                nc.vector.tensor_scalar(po[:], ee[:], rr[:], None, op0=ALU.mult)
                nc.sync.dma_start(out_ap, po[:])

            p2apart = acts.tile([NB, ANS], F32, tag="p2apart")
            M2 = ph.tile([128, 8, NB + 2, LN], BF16, tag="M2")
            nc.vector.memset(M2[:, :, 0, :], 0.0)
            nc.vector.memset(M2[:, :, NB + 1, :], 0.0)
            with tc.tile_pool(name="pp2g", bufs=1) as pp:
                gi_p2g = pp.tile([128, 2, 12, NB, LN], BF16, tag="gi_p2g")
                gib_p = load_gib("p2g", pp)
                # per-k wih tiles: gi matmuls on chunk 0 start after 1/4 of
                # the weight DMA instead of all of it
                kin_p, kc_p = CFG["p2g"]["kin"], CFG["p2g"]["kc"]
                gcp = 3 * kc_p
                wk = {}
                for d in (0, 1):
                    woff, wshp = PK_D16[f"p2g_wih{d}"]
                    wb = pk16[woff:woff + int(np.prod(wshp))].rearrange(
                        "(p x) -> p x", p=128)
                    for k in range(kin_p):
                        t = pp.tile([128, gcp * 128], BF16, tag=f"p2gw{d}_{k}")
                        nc.sync.dma_start(
                            t[:], wb[:, k * gcp * 128:(k + 1) * gcp * 128])
                        wk[(d, k)] = t[:].rearrange("p (g n) -> p g n", n=128)

                class WV:
                    def __init__(self, d):
                        self.d = d
                    def __getitem__(self, idx):
                        _, k, g, _ = idx
                        return wk[(self.d, k)][:, g, :]
                wv_p = [WV(0), WV(1)]
                with tc.tile_pool(name="psgi_p2g", bufs=4, space="PSUM") as psp:
                    for d in (0, 1):
                        gi_phase("p2g", lambda k: M[:, k, 1:NB + 1, :], LN,
                                 gi_p2g, psp, gib_p, wv_p, dirs=(d,))
                # p1 partial + p2's gsum part, issued after the gi matmuls so
                # the PE starts gi immediately; the AllReduce AND p1's softmax
                # ride under the p2g recurrence; only the m2sum part of p2 is
                # left for the tail.
                head_partial("p1_wT", "p1_bd", 20, 0, [gsum, msum], p1part, "1")
                head_partial("p2_wT", "p2_bd", 24, 0, [gsum], p2apart, "2a")
                allreduce([p1part[:], p2apart[:]], ALU.add)
                tap("p1post", p1part)
                head_post(p1part,
                          out2[0:NB * ANS].rearrange("(b a) -> b a", b=NB), "1")
                whh_v, bhnr_sb = load_whh("p2g", pp)
                with tc.tile_pool(name="psbh_p2g", bufs=1, space="PSUM") as psb:
                    bc_p2g = make_bhn_bc("p2g", pp, psb, bhnr_sb, LN)

                def m2sum_part(b_lo, b_hi):
                    # rows [b_lo+1, b_hi] of M2 are complete: reduce them
                    # while the recurrence continues on the outer rows
                    for k in range(8):
                        nc.vector.tensor_reduce(
                            m2sum[:, k, b_lo:b_hi],
                            M2[:, k, 1 + b_lo:1 + b_hi, :],
                            axis=AX.X, op=ALU.add)

                with tc.tile_pool(name="psrec_p2g", bufs=2, space="PSUM") as psr, \
                     tc.tile_pool(name="cell_p2g", bufs=2) as cellp:
                    recur_steps("p2g", whh_v, bc_p2g, gi_p2g, M2, LN, psr,
                                cellp, range(0, 21))
                    m2sum_part(11, 21)
                    recur_steps("p2g", whh_v, bc_p2g, gi_p2g, M2, LN, psr,
                                cellp, range(21, 26))
                    m2sum_part(6, 11)
                    m2sum_part(21, 26)
                    recur_steps("p2g", whh_v, bc_p2g, gi_p2g, M2, LN, psr,
                                cellp, range(26, NB))
                    m2sum_part(0, 6)
                    m2sum_part(26, NB)
            tap("M2", M2)
            tap("m2sum", m2sum)

            # p2's m2sum part: small tail AllReduce, then combine + softmax
            head_partial("p2_wT", None, 24, 16, [m2sum], p2part, "2b")
            allreduce([p2part[:]], ALU.add)
            nc.vector.tensor_tensor(p2part[:], p2part[:], p2apart[:],
                                    op=ALU.add)
            head_post(p2part,
                      out2[NB * ANS:2 * NB * ANS].rearrange("(b a) -> b a", b=NB),
                      "2")

    lp.__exit__(None, None, None)
    _split_excess_waits(nc)
    return nc


# ---------------------------------------------------------------- host prep
def _fm_stat(wT, kin, gc):
    din, dout = wT.shape
    assert din == kin * 128 and dout == gc * 128, (wT.shape, kin, gc)
    return np.ascontiguousarray(
        wT.reshape(kin, 128, gc, 128).transpose(1, 0, 2, 3).reshape(128, -1)
    ).astype(BF)


def _prep_params(i):
    out = {}
    for name in CFG:
        kin, kc = CFG[name]["kin"], CFG[name]["kc"]
        gc = 3 * kc
        wih = np.asarray(i[f"{name}_Wih"], np.float32)
        whh = np.asarray(i[f"{name}_Whh"], np.float32)
        bih = np.asarray(i[f"{name}_bih"], np.float32)
        bhh = np.asarray(i[f"{name}_bhh"], np.float32)
        for d in range(2):
            out[f"{name}_wih{d}"] = _fm_stat(wih[d].T, kin, gc)
        out[f"{name}_whh"] = np.stack(
            [_fm_stat(whh[d].T, kc, gc) for d in range(2)], axis=1
        ).reshape(128, -1)
        H = kc * 128
        gib = np.zeros((128, 2, gc), np.float32)
        bhnr = np.zeros((1, 2, kc * 128), np.float32)
        for d in range(2):
            v = bih[d].copy()
            v[:2 * H] += bhh[d][:2 * H]
            gib[:, d, :] = v.reshape(gc, 128).T
            bhnr[0, d, :] = bhh[d][2 * H:]
        out[f"{name}_gib"] = gib.reshape(128, -1)
        out[f"{name}_bhnr"] = bhnr.reshape(1, -1).astype(BF)

    W = np.asarray(i["W"], np.float32)
    out["w123"] = np.ascontiguousarray(np.stack(
        [W[0:512].reshape(4, 128).T, W[512:1024].reshape(4, 128).T,
         W[1024:1536].reshape(4, 128).T], axis=-1)).reshape(128, -1).astype(
        np.float32)

    def headw(w, nchunk):
        wT = np.asarray(w, np.float32).T
        K = wT.shape[0]
        arr = np.zeros((128, nchunk, ANS), np.float32)
        arr[:, :K // 128, :] = wT.reshape(K // 128, 128, ANS).transpose(1, 0, 2)
        return arr.reshape(128, -1).astype(BF)

    out["p1_wT"] = headw(i["p1_w"], 20)
    out["p2_wT"] = headw(i["p2_w"], 24)
    out["p1_bd"] = (np.asarray(i["p1_b"], np.float32)[None, :] / NCORES).astype(BF)
    out["p2_bd"] = (np.asarray(i["p2_b"], np.float32)[None, :] / NCORES).astype(BF)
    out["ident"] = np.eye(128, dtype=np.float32).astype(BF)
    return out


def _prep_x(embd_ctx, embd_q):
    xc = np.asarray(embd_ctx, np.float32)
    xq = np.asarray(embd_q, np.float32)
    per_core = []
    for c in range(NCORES):
        x = np.zeros((NB, W_CQ, 256), np.float32)
        x[:, 0:LN, :] = xc[:, c * LN:(c + 1) * LN, :]
        x[:, LN:LN + J, :] = xq
        xf = x.transpose(2, 0, 1)
        per_core.append(np.ascontiguousarray(
            xf.reshape(2, 128, NB, W_CQ).transpose(1, 0, 2, 3)).astype(BF))
    return per_core


_BUILD_CACHE = {}

def _get_nc(taps=()):
    key = tuple(taps)
    if key not in _BUILD_CACHE:
        _BUILD_CACHE[key] = build_nc(key)
    return _BUILD_CACHE[key]


def make_in_maps(inputs):
    params = _prep_params(inputs)
    xs = _prep_x(inputs["embd_ctx"], inputs["embd_q"])

    pk32 = np.zeros(PK_N32, np.float32)
    for key, shp in PK_L32:
        off, _ = PK_D32[key]
        arr = params[key].astype(np.float32).reshape(-1)
        assert arr.size == int(np.prod(shp)), (key, arr.size, shp)
        pk32[off:off + arr.size] = arr

    pk16_shared = np.zeros(PK_N16, BF)
    for key, shp in PK_L16:
        if key == "x_all":
            continue
        off, _ = PK_D16[key]
        arr = params[key].astype(BF).reshape(-1)
        assert arr.size == int(np.prod(shp)), (key, arr.size, shp)
        pk16_shared[off:off + arr.size] = arr

    xoff, xshp = PK_D16["x_all"]
    xn = int(np.prod(xshp))
    in_maps = []
    for c in range(NCORES):
        pk16 = pk16_shared.copy()
        pk16[xoff:xoff + xn] = xs[c].reshape(-1)
        in_maps.append({"pk16": pk16, "pk32": pk32})
    return in_maps


def kernel(**inputs):
    nc = _get_nc()
    in_maps = make_in_maps(inputs)
    res = run_bass_kernel_spmd(nc, in_maps, core_ids=list(range(NCORES))).results
    o = np.asarray(res[0]["out2"], np.float32).reshape(2, NB, ANS)
    return o[0], o[1]
